# revision 68
# baseline (speedup 1.0000x reference)
"""Trainium2 Bass kernel for MultiHeadSelfAttention (RMSNorm + QKV + causal SDPA + out-proj).

Sharding: 8 cores = batch(2) x head-groups(4).  Each core handles one batch
element and 4 of the 16 heads; the out-projection is computed per-core over
its local 512-wide d-slice and the 4 partial [T, D] outputs per batch are
summed on the host.

Device-side layout choices (all matmuls are float32r, full PE speed at N=512):
  - x is fed transposed (xT [D, T]) so the d-contraction sits on partitions.
  - Q^T/K^T are produced in [dh, t] layout (directly usable by the score
    matmul); V in [t, e] layout (directly usable as AV lhsT).
  - Scores are computed transposed, St[k, q] = K @ Q^T, so exp+mask feed the
    AV matmul with no transposes anywhere.  Softmax denominator comes from a
    ones-row matmul; division is deferred to the [dh, q] attention output.
  - RMSNorm: norm_weight is folded into the QKV weights; the per-token
    rsqrt(mean(x^2)) scale is folded into Q (via a broadcast multiply) and V
    (per-partition scale), and into the exp() input scale for K.
"""

import sys

sys.path.insert(0, '/opt/trn_rl_repo')

import numpy as np

import concourse.bass as bass  # noqa: F401  (import order matters)
from concourse import bacc
import concourse.mybir as mybir
import concourse.tile as tile
from concourse.bass_utils import run_bass_kernel_spmd

B, T, D = 2, 2048, 2048
H_TOT, H_LOC, DH = 16, 4, 128
EL = H_LOC * DH            # 512: local q/k/v width
ND = D // 128              # 16 d-tiles
NT = T // 128              # 16 t-tiles
CH = 512                   # token chunk
NCH = T // CH              # 4 chunks
QT = CH // 128             # 4 q-tiles per chunk
EPS = 1e-6
F32 = mybir.dt.float32
F32R = mybir.dt.float32r
MULT = mybir.AluOpType.mult


def _build():
    nc = bacc.Bacc("TRN2")
    xT = nc.dram_tensor("xT", [D, T], F32R, kind="ExternalInput")
    wqkT = nc.dram_tensor("wqkT", [D, 2 * EL], F32R, kind="ExternalInput")
    wvT = nc.dram_tensor("wvT", [D, EL], F32R, kind="ExternalInput")
    woutT = nc.dram_tensor("woutT", [EL, D], F32R, kind="ExternalInput")
    nw = nc.dram_tensor("nw", [1, D], F32, kind="ExternalInput")
    mask = nc.dram_tensor("mask", [128, 128], F32R, kind="ExternalInput")
    ones_in = nc.dram_tensor("ones_in", [128, 32], F32R, kind="ExternalInput")
    outT = nc.dram_tensor("outT", [D, T], F32, kind="ExternalOutput")

    with tile.TileContext(nc) as tc:
        with tc.tile_pool(name="misc", bufs=1) as misc, \
             tc.tile_pool(name="dram", bufs=1, space="DRAM") as dramp:
            nw_col = misc.tile([128, ND], F32)
            nc.sync.dma_start(nw_col[:], nw.ap().rearrange("o (k p) -> p (o k)", p=128))
            ones = misc.tile([128, 32], F32R)
            nc.sync.dma_start(ones[:], ones_in[:, :])
            eps_sb = misc.tile([1, 1], F32)
            nc.gpsimd.memset(eps_sb[:], EPS)
            mask_sb = misc.tile([128, 128], F32R)
            nc.sync.dma_start(mask_sb[:], mask[:, :])
            s_row = misc.tile([1, T], F32)
            s_col = misc.tile([128, NT], F32)

            qT_d = dramp.tile([H_LOC, 128, T], F32R)
            kT_d = dramp.tile([H_LOC, 128, T], F32R)
            v_d = dramp.tile([T, EL], F32R)
            s_d = dramp.tile([1, T], F32)

            # ---------------- Phase A: RMSNorm stats + QKV projection ----------------
            with tc.tile_pool(name="wqkv", bufs=1) as wp, \
                 tc.tile_pool(name="xa", bufs=2) as xa_p, \
                 tc.tile_pool(name="pa_sb", bufs=2) as pa_sb, \
                 tc.tile_pool(name="pa_ps", bufs=2, space="PSUM") as pa_ps, \
                 tc.tile_pool(name="pa_ps1", bufs=1, space="PSUM") as pa_ps1:
                wqk_sb = wp.tile([128, ND, 2 * EL], F32R)
                wv_sb = wp.tile([128, ND, EL], F32R)
                xc0 = xa_p.tile([128, ND, CH], F32R, tag="xc")
                for kd in range(ND):
                    nc.sync.dma_start(xc0[:, kd, :], xT[kd * 128:(kd + 1) * 128, 0:CH])
                for kd in range(ND):
                    nc.sync.dma_start(wqk_sb[:, kd, :], wqkT[kd * 128:(kd + 1) * 128, :])
                    nc.vector.tensor_scalar_mul(wqk_sb[:, kd, :], wqk_sb[:, kd, :], nw_col[:, kd:kd + 1])
                for kd in range(ND):
                    nc.sync.dma_start(wv_sb[:, kd, :], wvT[kd * 128:(kd + 1) * 128, :])
                    nc.vector.tensor_scalar_mul(wv_sb[:, kd, :], wv_sb[:, kd, :], nw_col[:, kd:kd + 1])

                for c in range(NCH):
                    if c == 0:
                        xc = xc0
                    else:
                        xc = xa_p.tile([128, ND, CH], F32R, tag="xc")
                        for kd in range(ND):
                            nc.sync.dma_start(xc[:, kd, :], xT[kd * 128:(kd + 1) * 128, c * CH:(c + 1) * CH])
                    # sum of squares over d via ones-matmul (sq producers
                    # alternate ACT/DVE so neither engine serializes the chain)
                    ssq = pa_ps1.tile([1, CH], F32, tag="ssq")
                    for kd in range(ND):
                        sq = pa_sb.tile([128, CH], F32R, tag="sq", bufs=4)
                        if kd % 2 == 0:
                            nc.scalar.square(sq[:], xc[:, kd, :])
                        else:
                            nc.vector.tensor_tensor(sq[:], xc[:, kd, :], xc[:, kd, :], MULT)
                        nc.tensor.matmul(ssq[:], ones[:, 0:1], sq[:], start=(kd == 0), stop=(kd == ND - 1))
                    srow_c = s_row[0:1, c * CH:(c + 1) * CH]
                    tmp_s = pa_sb.tile([1, CH], F32, tag="tmp_s")
                    nc.scalar.activation(tmp_s[:], ssq[:], mybir.ActivationFunctionType.Sqrt,
                                         bias=eps_sb[:], scale=1.0 / D)
                    nc.vector.reciprocal(srow_c, tmp_s[:])
                    nc.sync.dma_start(s_d[0:1, c * CH:(c + 1) * CH], srow_c)
                    nc.sync.dma_start(s_col[:, c * QT:(c + 1) * QT],
                                      s_d[0:1, c * CH:(c + 1) * CH].rearrange("o (j p) -> p (o j)", p=128))
                    sb_c = pa_sb.tile([128, CH], F32, tag="sb_c")
                    nc.gpsimd.partition_broadcast(sb_c[:], srow_c)
                    # K projection: the MMs don't wait on stats; the ksc scale
                    # consumer (DVE) picks up sb_c when the stats chain lands.
                    for et in range(4, 8):
                        qk_ps = pa_ps.tile([128, CH], F32, tag="qk_ps", bufs=5)
                        for kd in range(ND):
                            nc.tensor.matmul(qk_ps[:], wqk_sb[:, kd, et * 128:(et + 1) * 128],
                                             xc[:, kd, :], start=(kd == 0), stop=(kd == ND - 1))
                        ksc = pa_sb.tile([128, CH], F32R, tag="qsc")
                        nc.vector.tensor_tensor(ksc[:], qk_ps[:], sb_c[:], MULT)
                        nc.sync.dma_start(kT_d[et - 4, :, c * CH:(c + 1) * CH], ksc[:])
                    # sum of squares over d via ones-matmul

                    for et in range(4):
                        qk_ps = pa_ps.tile([128, CH], F32, tag="qk_ps", bufs=5)
                        for kd in range(ND):
                            nc.tensor.matmul(qk_ps[:], wqk_sb[:, kd, et * 128:(et + 1) * 128],
                                             xc[:, kd, :], start=(kd == 0), stop=(kd == ND - 1))
                        qsc = pa_sb.tile([128, CH], F32R, tag="qsc")
                        nc.vector.tensor_tensor(qsc[:], qk_ps[:], sb_c[:], MULT)
                        nc.sync.dma_start(qT_d[et, :, c * CH:(c + 1) * CH], qsc[:])

                    for tt in range(QT):
                        j = c * QT + tt
                        v_ps = pa_ps.tile([128, CH], F32, tag="v_ps")
                        for kd in range(ND):
                            nc.tensor.matmul(v_ps[:], xc[:, kd, tt * 128:(tt + 1) * 128],
                                             wv_sb[:, kd, :], start=(kd == 0), stop=(kd == ND - 1))
                        vsc = pa_sb.tile([128, CH], F32R, tag="vsc")
                        nc.vector.tensor_scalar_mul(vsc[:], v_ps[:], s_col[:, j:j + 1])
                        nc.sync.dma_start(v_d[j * 128:(j + 1) * 128, :], vsc[:])

            # ---------------- Phase B: causal attention + out-projection ----------------
            with tc.tile_pool(name="kv", bufs=1) as kv_p, \
                 tc.tile_pool(name="pb_sb", bufs=3) as pb_sb, \
                 tc.tile_pool(name="pb_m", bufs=2) as pb_m, \
                 tc.tile_pool(name="pb_ps", bufs=2, space="PSUM") as pb_ps, \
                 tc.tile_pool(name="pb_ps1", bufs=1, space="PSUM") as pb_ps1:
                wout_sb = kv_p.tile([128, H_LOC, D], F32R)
                kT_sb = kv_p.tile([128, H_LOC, T], F32R)
                v_sb = kv_p.tile([128, NT, EL], F32R)
                # chunk-ordered loads: earliest-needed tiles first, wout last
                q_tiles = []
                for cc in range(NCH):
                    q_sb = pb_m.tile([128, H_LOC, CH], F32R, tag="q_sb", name=f"q_sb_{cc}")
                    for h in range(H_LOC):
                        nc.sync.dma_start(q_sb[:, h, :], qT_d[h, :, cc * CH:(cc + 1) * CH])
                        nc.sync.dma_start(kT_sb[:, h, cc * CH:(cc + 1) * CH],
                                          kT_d[h, :, cc * CH:(cc + 1) * CH])
                    for tt in range(QT):
                        j = cc * QT + tt
                        nc.sync.dma_start(v_sb[:, j, :], v_d[j * 128:(j + 1) * 128, :])
                    q_tiles.append(q_sb)
                for dl in range(H_LOC):
                    nc.sync.dma_start(wout_sb[:, dl, :], woutT[dl * 128:(dl + 1) * 128, :])

                SC = float(1.0 / np.sqrt(DH))
                pending = []

                def _emit_outproj(c, y_sb):
                    for eo in range(NT):
                        o_ps = pb_ps.tile([128, CH], F32, tag="o_ps", bufs=2)
                        for dl in range(H_LOC):
                            nc.tensor.matmul(o_ps[:], wout_sb[:, dl, eo * 128:(eo + 1) * 128],
                                             y_sb[:, dl, :], start=(dl == 0), stop=(dl == H_LOC - 1))
                        o_sb = pb_sb.tile([128, CH], F32, tag="o_sb", bufs=4)
                        nc.vector.tensor_copy(o_sb[:], o_ps[:])
                        nc.sync.dma_start(outT[eo * 128:(eo + 1) * 128, c * CH:(c + 1) * CH], o_sb[:])

                for c in range(NCH):
                    q_sb = q_tiles[c]
                    y_sb = pb_m.tile([128, H_LOC, CH], F32R, tag="y_sb", bufs=3)
                    jmax = (c + 1) * QT
                    for h in range(H_LOC):
                        y_ps = pb_ps.tile([128, CH], F32, tag="y_ps", bufs=1)
                        z_ps = pb_ps1.tile([1, CH], F32, tag="z_ps", bufs=1)
                        for j in range(jmax):
                            # q-columns < off are fully masked for this k-tile: skip them
                            off = (j - c * QT) * 128 if j >= c * QT else 0
                            st_ps = pb_ps.tile([128, CH], F32, tag="st_ps", bufs=4)
                            nc.tensor.matmul(st_ps[:, off:], kT_sb[:, h, j * 128:(j + 1) * 128],
                                             q_sb[:, h, off:], start=True, stop=True)
                            pt = pb_sb.tile([128, CH], F32R, tag="pt", bufs=7)
                            nc.scalar.activation(pt[:, off:], st_ps[:, off:],
                                                 mybir.ActivationFunctionType.Exp, scale=SC)
                            if j >= c * QT:
                                nc.vector.tensor_tensor(pt[:, off:off + 128], pt[:, off:off + 128],
                                                        mask_sb[:], MULT)
                            nc.tensor.matmul(z_ps[0:1, off:], ones[:, 0:1], pt[:, off:],
                                             start=(j == 0), stop=(j == jmax - 1))
                            nc.tensor.matmul(y_ps[:, off:], v_sb[:, j, h * 128:(h + 1) * 128],
                                             pt[:, off:], start=(j == 0), stop=(j == jmax - 1))
                        rz = pb_m.tile([1, CH], F32, tag="rz")
                        nc.vector.reciprocal(rz[:], z_ps[0:1, :])
                        rzb = pb_m.tile([128, CH], F32, tag="rzb", bufs=3)
                        nc.gpsimd.partition_broadcast(rzb[:], rz[:])
                        nc.vector.tensor_tensor(y_sb[:, h, :], y_ps[:], rzb[:], MULT)
                    _emit_outproj(c, y_sb)
    nc.finalize()
    return nc


_BUILT = None


def _get_nc():
    global _BUILT
    if _BUILT is None:
        _BUILT = _build()
    return _BUILT


def _make_in_maps(x, norm_weight, w_qkv, w_out):
    x = np.asarray(x, dtype=np.float32)
    norm_weight = np.asarray(norm_weight, dtype=np.float32)
    w_qkv = np.asarray(w_qkv, dtype=np.float32)
    w_out = np.asarray(w_out, dtype=np.float32)
    mask_ut = np.triu(np.ones((128, 128), dtype=np.float32))
    nw_row = np.ascontiguousarray(norm_weight.reshape(1, D))
    in_maps = []
    for core in range(8):
        b, g = divmod(core, 4)
        sl = slice(EL * g, EL * (g + 1))
        wq = w_qkv[0 * D:1 * D][sl]
        wk = w_qkv[1 * D:2 * D][sl]
        wv = w_qkv[2 * D:3 * D][sl]
        in_maps.append({
            "xT": np.ascontiguousarray(x[b].T),
            "wqkT": np.ascontiguousarray(np.concatenate([wq, wk], axis=0).T),
            "wvT": np.ascontiguousarray(wv.T),
            "woutT": np.ascontiguousarray(w_out[:, sl].T),
            "nw": nw_row,
            "mask": mask_ut,
            "ones_in": np.ones((128, 32), dtype=np.float32),
        })
    return in_maps


def _gather(results):
    out = np.zeros((B, T, D), dtype=np.float32)
    for core in range(8):
        b, _g = divmod(core, 4)
        out[b] += results[core]["outT"].T
    return out


def run(x, norm_weight, w_qkv, w_out, trace=False):
    in_maps = _make_in_maps(x, norm_weight, w_qkv, w_out)
    if trace:
        try:
            res = run_bass_kernel_spmd(_get_nc(), in_maps, list(range(8)), trace=True)
            return _gather(res.results), res
        except Exception:
            pass  # NTFF hook unavailable under this axon client; run untraced
    res = run_bass_kernel_spmd(_get_nc(), in_maps, list(range(8)), trace=False)
    return _gather(res.results), res


def kernel(x, norm_weight, w_qkv, w_out):
    out, _res = run(x, norm_weight, w_qkv, w_out)
    return out
